# revision 1
# baseline (speedup 1.0000x reference)
"""Trainium2 Bass kernel for nn_Att_23313082483285 (GNN message passing).

Strategy: partition agent nodes across 8 cores (8192 each). Host routes each
edge to the core owning its destination agent (hi), groups edges by 128-node
block, splits each block's edges by wi < 32768 (dma_gather idx is int16), and
pads each (block, half) to a fixed tile count so all cores run one SPMD
program. On device, per 128-edge tile: edge MLP via PE matmuls in
[edge, feature] layout (activations transposed on PE as needed), GroupNorm via
bn_stats (free-dim stats, per-partition scale/bias on ScalarE), q-branch
hoisted to node level (gn(agts@q_w) then @Wq computed once per node, gathered
per edge), and scatter-add via one-hot matmuls accumulating in PSUM per node
block. No collectives: each core owns its output rows.
"""
import sys
sys.path.insert(0, '/opt/trn_rl_repo')

import numpy as np
import ml_dtypes
from contextlib import ExitStack

from concourse import bass, mybir, tile
import concourse.bacc as bacc
from concourse.bass_utils import run_bass_kernel_spmd
from concourse.masks import make_identity

bf16 = ml_dtypes.bfloat16
P = 128
N_AGT = 65536
N_CTX = 65536
E = 400000
D = 128
EPS = 1e-5
NCORES = 8
NPC = N_AGT // NCORES          # 8192 nodes per core
NBLK = NPC // P                # 64 blocks per core
CTX_HALF = 32768
G_TILES = 16                   # tiles per dma_gather op (2048 idxs)

f32 = mybir.dt.float32
bft = mybir.dt.bfloat16
i16 = mybir.dt.int16


def _wrap16(flat_idx):
    """dma_gather idx layout: [16, n/16] with idx[c,k]=flat[k*16+c], tiled x8."""
    w = flat_idx.reshape(-1, 16).T.astype(np.int16)
    return np.ascontiguousarray(np.tile(w, (8, 1)))


def _host_prep(agts, ctx, agt_ctrs, ctx_ctrs, hi, wi, weights):
    """Route/pad edges per core; build all per-core device input arrays."""
    hi = np.asarray(hi).astype(np.int64)
    wi = np.asarray(wi).astype(np.int64)
    agts = np.asarray(agts, dtype=np.float32)
    ctx = np.asarray(ctx, dtype=np.float32)
    d_all = (np.asarray(agt_ctrs, np.float32)[hi]
             - np.asarray(ctx_ctrs, np.float32)[wi])          # [E, 2]

    core = hi // NPC
    blk_global = hi // P          # 0..511
    lidx_all = hi % P
    is_hi = wi >= CTX_HALF

    # per (global block, half) edge lists
    lists = [[[] for _ in range(2)] for _ in range(N_AGT // P)]
    order = np.argsort(blk_global * 2 + is_hi, kind='stable')
    for e in order:
        lists[blk_global[e]][1 if is_hi[e] else 0].append(e)

    t_lo = max(max((len(l[0]) for l in lists), default=0), 1)
    t_hi = max(max((len(l[1]) for l in lists), default=0), 1)
    T_LO = -(-(-(-t_lo // P)) // 4) * 4   # round tiles up to multiple of 4
    T_HI = -(-(-(-t_hi // P)) // 4) * 4
    T_B = T_LO + T_HI
    NT = NBLK * T_B               # tiles per core
    EPAD = NT * P

    cores = []
    for k in range(NCORES):
        dT4 = np.zeros((4, EPAD), np.float32)
        lidxT = np.full((P, NT), -1.0, np.float32)
        qflat = np.zeros(EPAD, np.int64)
        lo_flat = np.zeros(NBLK * T_LO * P, np.int64)
        hi_flat = np.zeros(NBLK * T_HI * P, np.int64)

        for b in range(NBLK):
            gb = k * NBLK + b
            for half, (TH, flat, base_t) in enumerate(
                    ((T_LO, lo_flat, 0), (T_HI, hi_flat, 0))):
                edges = lists[gb][half]
                assert len(edges) <= TH * P, (
                    f"block overflow core {k} blk {b} half {half}: "
                    f"{len(edges)} > {TH * P}")
                for j, e in enumerate(edges):
                    tl = j // P          # tile within this half
                    p = j % P
                    t = b * T_B + (0 if half == 0 else T_LO) + tl  # global tile
                    col = t * P + p
                    dT4[0, col] = d_all[e, 0]
                    dT4[1, col] = d_all[e, 1]
                    dT4[2, col] = 1.0
                    lidxT[p, t] = float(lidx_all[e])
                    qflat[col] = lidx_all[e] + (blk_global[e] % NBLK) * P
                    si = b * TH * P + tl * P + p   # slot in half-stream
                    flat[si] = wi[e] if half == 0 else wi[e] - CTX_HALF

        cores.append(dict(
            agtsT=np.ascontiguousarray(agts[k * NPC:(k + 1) * NPC].T.astype(bf16)),
            agts_res=np.ascontiguousarray(agts[k * NPC:(k + 1) * NPC]),
            dT4=dT4.astype(bf16),
            lidxT=lidxT.astype(bf16),
            qidx=_wrap16(qflat),
            widx_lo=_wrap16(lo_flat),
            widx_hi=_wrap16(hi_flat),
        ))

    shared = dict(ctx_bf=np.ascontiguousarray(ctx.astype(bf16)), **weights)
    return cores, shared, T_LO, T_HI


def _build_program(T_LO, T_HI):
    T_B = T_LO + T_HI
    NT = NBLK * T_B
    NST = NT // 4                 # super-tiles of 4 tiles
    N_LO = NBLK * T_LO            # lo tiles per core
    N_HI = NBLK * T_HI

    nc = bacc.Bacc("TRN2", target_bir_lowering=False, debug=False,
                   enable_asserts=False, num_devices=NCORES,
                   dynamic_dma_scratch_size=16384)

    def din(name, shape, dt):
        return nc.dram_tensor(name, list(shape), dt, kind="ExternalInput").ap()

    t_agtsT = din("agtsT", (P, NPC), bft)
    t_res = din("agts_res", (NPC, D), f32)
    t_ctx = din("ctx_bf", (N_CTX, D), bft)
    t_dT4 = din("dT4", (4, NT * P), bft)
    t_lidxT = din("lidxT", (P, NT), bft)
    t_qidx = din("qidx", (P, NT * P // 16), i16)
    t_wlo = din("widx_lo", (P, N_LO * P // 16), i16)
    t_whi = din("widx_hi", (P, N_HI * P // 16), i16)
    wnames = ["w1_aug", "dist_w2", "q_w", "Wd", "Wq", "Wc", "ctx_w2",
              "agt_w", "lin_w"]
    t_w = {n: din(n, (4, D) if n == "w1_aug" else (D, D), bft) for n in wnames}
    t_out = nc.dram_tensor("out", [NPC, D], f32, kind="ExternalOutput").ap()

    with tile.TileContext(nc) as tc, ExitStack() as ctx:
        const = ctx.enter_context(tc.tile_pool(name="const", bufs=1))
        big = ctx.enter_context(tc.tile_pool(name="big", bufs=1))
        dram = ctx.enter_context(tc.tile_pool(name="dram", bufs=1, space="DRAM"))
        sb = ctx.enter_context(tc.tile_pool(name="sb", bufs=4))
        gb = ctx.enter_context(tc.tile_pool(name="gb", bufs=3))
        ps = ctx.enter_context(tc.tile_pool(name="ps", bufs=8, space="PSUM"))
        psS = ps

        # ---------- constants ----------
        ident = const.tile([P, P], f32)
        make_identity(nc, ident[:])
        ident_bf = const.tile([P, P], bft)
        nc.vector.tensor_copy(ident_bf[:], ident[:])
        iota_i = const.tile([P, 4, P], mybir.dt.int32)
        nc.gpsimd.iota(iota_i[:], pattern=[[0, 4], [1, P]], base=0,
                       channel_multiplier=0)
        iota_bf = const.tile([P, 4, P], bft)
        nc.vector.tensor_copy(iota_bf[:], iota_i[:])
        eps_t = const.tile([P, 1], f32)
        nc.gpsimd.memset(eps_t[:], EPS)
        w_sb = {}
        for n in wnames:
            shp = [4, D] if n == "w1_aug" else [D, D]
            w_sb[n] = const.tile(shp, bft, name=f"w_{n}")
            nc.sync.dma_start(w_sb[n][:], t_w[n][:])

        # big resident tensors
        agtsT = big.tile([P, NPC], bft)
        nc.sync.dma_start(agtsT[:], t_agtsT[:])
        lidxT = big.tile([P, NT], bft)
        nc.sync.dma_start(lidxT[:], t_lidxT[:])
        qidx = big.tile([P, NT * P // 16], i16)
        nc.sync.dma_start(qidx[:], t_qidx[:])
        wlo = big.tile([P, N_LO * P // 16], i16)
        nc.sync.dma_start(wlo[:], t_wlo[:])
        whi = big.tile([P, N_HI * P // 16], i16)
        nc.sync.dma_start(whi[:], t_whi[:])
        node_accum = big.tile([P, NBLK, D], f32)

        Q2_dram = dram.tile([NPC, D], bft)

        # helper: GroupNorm stats from a [P,4,D] psum -> (rinv, mbias) [P,4]
        def gn_stats(psum_t, tag):
            bn6 = sb.tile([P, 4, 6], f32, tag=f"bn6{tag}")
            for c in range(4):
                nc.vector.bn_stats(bn6[:, c, :], psum_t[:, c, :])
            bn2 = sb.tile([P, 4, 2], f32, tag=f"bn2{tag}")
            for c in range(4):
                nc.vector.bn_aggr(bn2[:, c, :], bn6[:, c, :])
            sd = sb.tile([P, 4], f32, tag=f"sd{tag}")
            nc.scalar.activation(sd[:], bn2[:, :, 1],
                                 mybir.ActivationFunctionType.Sqrt,
                                 bias=eps_t[:])
            rinv = sb.tile([P, 4], f32, tag=f"ri{tag}")
            nc.vector.reciprocal(rinv[:], sd[:])
            mb = sb.tile([P, 4], f32, tag=f"mb{tag}")
            nc.vector.tensor_tensor(out=mb[:], in0=bn2[:, :, 0], in1=rinv[:],
                                    op=mybir.AluOpType.mult)
            nc.vector.tensor_scalar_mul(mb[:], mb[:], -1.0)
            return rinv, mb

        # ---------- phase 1: node-level precompute ----------
        # A1 = agts @ agt_w -> node_accum ; Q2 = (relu(gn(agts@q_w)) @ Wq) -> DRAM
        for g in range(NBLK // 4):        # 16 groups of 4 node chunks
            ps_a = ps.tile([P, 4, D], f32, space="PSUM", tag="pp")
            for c in range(4):
                j = g * 4 + c
                nc.tensor.matmul(ps_a[:, c, :], lhsT=agtsT[:, j * P:(j + 1) * P],
                                 rhs=w_sb["agt_w"][:], start=True, stop=True)
            nc.scalar.copy(node_accum[:, g * 4:(g + 1) * 4, :], ps_a[:])

            ps_q = ps.tile([P, 4, D], f32, space="PSUM", tag="pp")
            for c in range(4):
                j = g * 4 + c
                nc.tensor.matmul(ps_q[:, c, :], lhsT=agtsT[:, j * P:(j + 1) * P],
                                 rhs=w_sb["q_w"][:], start=True, stop=True)
            rinv, mb = gn_stats(ps_q, "n")
            qn = sb.tile([P, 4, D], bft, tag="qn")
            for c in range(4):
                nc.scalar.activation(qn[:, c, :], ps_q[:, c, :],
                                     mybir.ActivationFunctionType.Relu,
                                     bias=mb[:, c:c + 1], scale=rinv[:, c:c + 1])
            ps_t = ps.tile([P, 4, D], bft, space="PSUM", tag="pp")
            for c in range(4):
                nc.tensor.transpose(ps_t[:, c, :], qn[:, c, :], ident_bf[:])
            qnT = sb.tile([P, 4, D], bft, tag="qnT")
            nc.scalar.copy(qnT[:], ps_t[:])
            ps_q2 = ps.tile([P, 4, D], f32, space="PSUM", tag="pp")
            for c in range(4):
                nc.tensor.matmul(ps_q2[:, c, :], lhsT=qnT[:, c, :],
                                 rhs=w_sb["Wq"][:], start=True, stop=True)
            q2sb = sb.tile([P, 4, D], bft, tag="q2sb")
            nc.vector.tensor_copy(q2sb[:], ps_q2[:])
            nc.sync.dma_start(
                Q2_dram[g * 512:(g + 1) * 512, :].rearrange(
                    "(c p) f -> p c f", p=P),
                q2sb[:])

        # ---------- phase 2: edge pipeline ----------
        # per-tile metadata
        lo_ctr = 0
        hi_ctr = 0
        tmeta = []                    # (blk, pos, half, slice_in_half_stream)
        for t in range(NT):
            b, pos = divmod(t, T_B)
            if pos < T_LO:
                tmeta.append((b, pos, 0, lo_ctr)); lo_ctr += 1
            else:
                tmeta.append((b, pos, 1, hi_ctr)); hi_ctr += 1

        qa_bufs = {}
        lo_bufs = {}
        hi_bufs = {}
        dt_bufs = {}

        def issue_dt(gi):
            nt = min(G_TILES, NT - gi * G_TILES)
            buf = gb.tile([4, G_TILES * P], bft, tag="dt4")
            nc.sync.dma_start(buf[:, :nt * P],
                              t_dT4[:, gi * G_TILES * P:(gi * G_TILES + nt) * P])
            dt_bufs[gi] = buf

        def issue_qa(gi):
            nt = min(G_TILES, NT - gi * G_TILES)
            buf = gb.tile([P, G_TILES, D], bft, tag="qa")
            nc.gpsimd.dma_gather(
                out_ap=buf[:, :nt, :], in_ap=Q2_dram[:],
                idxs_ap=qidx[:, gi * G_TILES * 8:(gi * G_TILES + nt) * 8],
                num_idxs=nt * P, num_idxs_reg=nt * P, elem_size=D,
                single_packet=False)
            qa_bufs[gi] = buf

        def issue_w(gi, half):
            n_str, src, idxt, bufs, tag = (
                (N_LO, t_ctx[:CTX_HALF, :], wlo, lo_bufs, "clo") if half == 0
                else (N_HI, t_ctx[CTX_HALF:, :], whi, hi_bufs, "chi"))
            nt = min(G_TILES, n_str - gi * G_TILES)
            buf = gb.tile([P, 1, G_TILES * P], bft, tag=tag)
            nc.gpsimd.dma_gather(
                out_ap=buf[:, :, :nt * P], in_ap=src,
                idxs_ap=idxt[:, gi * G_TILES * 8:(gi * G_TILES + nt) * 8],
                num_idxs=nt * P, num_idxs_reg=nt * P, elem_size=D,
                transpose=True, single_packet=False)
            bufs[gi] = buf

        assert T_LO % 4 == 0 and T_HI % 4 == 0, (T_LO, T_HI)
        assert G_TILES % 4 == 0

        def st_stages(s):
            """Edge pipeline for super-tile s (4 tiles), yielded per stage so
            two STs can be emitted stage-interleaved (hides engine-hop
            latency: each engine always has the partner ST's op in flight)."""
            tiles = [4 * s + c for c in range(4)]
            gi0, off0 = divmod(4 * s, G_TILES)
            # stage 0: gathers / dT chunk loads
            if gi0 not in qa_bufs:
                issue_qa(gi0)
            if gi0 not in dt_bufs:
                issue_dt(gi0)
            for t in tiles:
                _b, _pos, half, si = tmeta[t]
                gi = si // G_TILES
                if half == 0 and gi not in lo_bufs:
                    issue_w(gi, 0)
                if half == 1 and gi not in hi_bufs:
                    issue_w(gi, 1)
            yield
            # stage 1: L1 (form B): y1T [f,512e] = w1_aug.T @ dT4
            ps_y1 = ps.tile([P, 4 * D], f32, space="PSUM", tag="pp")
            nc.tensor.matmul(ps_y1[:], lhsT=w_sb["w1_aug"][:],
                             rhs=dt_bufs[gi0][:, off0 * P:(off0 + 4) * P],
                             start=True, stop=True)
            yield
            # stage 2: relu -> r1T [f, 4x e] bf16
            r1T = sb.tile([P, 4 * D], bft, tag="r1T")
            nc.scalar.activation(r1T[:], ps_y1[:],
                                 mybir.ActivationFunctionType.Relu)
            yield
            # stage 3: L2
            ps2 = ps.tile([P, 4, D], f32, space="PSUM", tag="pp")
            for c in range(4):
                nc.tensor.matmul(ps2[:, c, :], lhsT=r1T[:, c * D:(c + 1) * D],
                                 rhs=w_sb["dist_w2"][:], start=True, stop=True)
            yield
            # stage 4: dist GN stats
            rinv, mb = gn_stats(ps2, "d")
            yield
            # stage 5: GN act -> h
            h = sb.tile([P, 4, D], bft, tag="h")
            for c in range(4):
                nc.scalar.activation(h[:, c, :], ps2[:, c, :],
                                     mybir.ActivationFunctionType.Relu,
                                     bias=mb[:, c:c + 1], scale=rinv[:, c:c + 1])
            yield
            # stage 6: T(h)
            psT2 = ps.tile([P, 4, D], bft, space="PSUM", tag="pp")
            for c in range(4):
                nc.tensor.transpose(psT2[:, c, :], h[:, c, :], ident_bf[:])
            yield
            # stage 7: hT copy
            hT = sb.tile([P, 4, D], bft, tag="hT")
            nc.vector.tensor_copy(hT[:], psT2[:])
            yield
            # stage 8: C1 = h@Wd + ctx[wi]@Wc + Q2[hi]
            ps3 = ps.tile([P, 4, D], f32, space="PSUM", tag="pp")
            for c, t in enumerate(tiles):
                _b, _pos, half, si = tmeta[t]
                gi, off = divmod(si, G_TILES)
                cbuf = lo_bufs[gi] if half == 0 else hi_bufs[gi]
                qgi, qoff = divmod(t, G_TILES)
                nc.tensor.matmul(ps3[:, c, :], lhsT=hT[:, c, :],
                                 rhs=w_sb["Wd"][:], start=True, stop=False)
                nc.tensor.matmul(ps3[:, c, :],
                                 lhsT=cbuf[:, 0, off * P:(off + 1) * P],
                                 rhs=w_sb["Wc"][:], start=False, stop=False)
                nc.tensor.matmul(ps3[:, c, :], lhsT=ident_bf[:],
                                 rhs=qa_bufs[qgi][:, qoff, :],
                                 start=False, stop=True)
            yield
            # stage 9: ctx GN stats
            rinv3, mb3 = gn_stats(ps3, "c")
            yield
            # stage 10: GN act -> g
            gt = sb.tile([P, 4, D], bft, tag="gt")
            for c in range(4):
                nc.scalar.activation(gt[:, c, :], ps3[:, c, :],
                                     mybir.ActivationFunctionType.Relu,
                                     bias=mb3[:, c:c + 1],
                                     scale=rinv3[:, c:c + 1])
            yield
            # stage 11: T(g)
            psT3 = ps.tile([P, 4, D], bft, space="PSUM", tag="pp")
            for c in range(4):
                nc.tensor.transpose(psT3[:, c, :], gt[:, c, :], ident_bf[:])
            yield
            # stage 12: gT copy
            gT = sb.tile([P, 4, D], bft, tag="gT")
            nc.vector.tensor_copy(gT[:], psT3[:])
            yield
            # stage 13: C2 -> m
            ps4 = ps.tile([P, 4, D], f32, space="PSUM", tag="pp")
            for c in range(4):
                nc.tensor.matmul(ps4[:, c, :], lhsT=gT[:, c, :],
                                 rhs=w_sb["ctx_w2"][:], start=True, stop=True)
            yield
            # stage 14: m copy + one-hot
            m_sb = sb.tile([P, 4, D], bft, tag="m_sb")
            nc.scalar.copy(m_sb[:], ps4[:])
            oh = sb.tile([P, 4, D], bft, tag="oh")
            nc.vector.tensor_tensor(
                out=oh[:], in0=iota_bf[:],
                in1=lidxT[:, 4 * s:4 * s + 4][:, :, None].to_broadcast(
                    [P, 4, P]),
                op=mybir.AluOpType.is_equal)
            yield
            # stage 15: scatter (all 4 tiles hit one node block)
            b0 = tmeta[tiles[0]][0]
            assert all(tmeta[t][0] == b0 for t in tiles)
            ps_s = psS.tile([P, D], f32, space="PSUM", tag="pp")
            for c in range(4):
                nc.tensor.matmul(ps_s[:], lhsT=oh[:, c, :],
                                 rhs=m_sb[:, c, :],
                                 start=(c == 0), stop=(c == 3))
            yield
            # stage 16: accumulate into node_accum
            nc.vector.tensor_tensor(
                out=node_accum[:, b0, :], in0=node_accum[:, b0, :],
                in1=ps_s[:], op=mybir.AluOpType.add)
            yield

        ILV = 8
        for grp in range(NST // ILV):
            gens = [st_stages(ILV * grp + j) for j in range(ILV)]
            alive = True
            while alive:
                alive = False
                for g_ in gens:
                    try:
                        next(g_)
                        alive = True
                    except StopIteration:
                        pass

        # ---------- phase 3: node finale ----------
        for g in range(NBLK // 4):
            rinv, mb = gn_stats(node_accum[:, g * 4:(g + 1) * 4, :], "f")
            o1 = sb.tile([P, 4, D], bft, tag="o1")
            for c in range(4):
                nc.scalar.activation(o1[:, c, :],
                                     node_accum[:, g * 4 + c, :],
                                     mybir.ActivationFunctionType.Relu,
                                     bias=mb[:, c:c + 1], scale=rinv[:, c:c + 1])
            ps_t = ps.tile([P, 4, D], bft, space="PSUM", tag="pp")
            for c in range(4):
                nc.tensor.transpose(ps_t[:, c, :], o1[:, c, :], ident_bf[:])
            o1T = sb.tile([P, 4, D], bft, tag="o1T")
            nc.vector.tensor_copy(o1T[:], ps_t[:])
            ps_l = ps.tile([P, 4, D], f32, space="PSUM", tag="pp")
            for c in range(4):
                nc.tensor.matmul(ps_l[:, c, :], lhsT=o1T[:, c, :],
                                 rhs=w_sb["lin_w"][:], start=True, stop=True)
            rinv, mb = gn_stats(ps_l, "l")
            o2 = sb.tile([P, 4, D], f32, tag="o2")
            for c in range(4):
                nc.scalar.activation(o2[:, c, :], ps_l[:, c, :],
                                     mybir.ActivationFunctionType.Identity,
                                     bias=mb[:, c:c + 1], scale=rinv[:, c:c + 1])
            res_sb = sb.tile([P, 4, D], f32, tag="res_sb")
            nc.sync.dma_start(
                res_sb[:],
                t_res[g * 512:(g + 1) * 512, :].rearrange(
                    "(c p) f -> p c f", p=P))
            fin = sb.tile([P, 4, D], f32, tag="fin")
            nc.vector.tensor_tensor(out=fin[:], in0=o2[:], in1=res_sb[:],
                                    op=mybir.AluOpType.add)
            nc.vector.tensor_scalar_max(fin[:], fin[:], 0.0)
            nc.sync.dma_start(
                t_out[g * 512:(g + 1) * 512, :].rearrange(
                    "(c p) f -> p c f", p=P),
                fin[:])

    nc.compile()
    return nc


_cached = {}
_extra_run_kwargs = {}
_last_results = None


def run_traced(inputs):
    """Run once more with NTFF tracing; returns BassKernelResults."""
    global _extra_run_kwargs
    _extra_run_kwargs = dict(trace=True)
    try:
        kernel(**inputs)
    finally:
        _extra_run_kwargs = {}
    return _last_results


def kernel(agts, ctx, agt_ctrs, ctx_ctrs, hi, wi,
           dist_w1, dist_b1, dist_w2, dist_gw, dist_gb,
           q_w, q_gw, q_gb,
           ctx_w1, ctx_gw, ctx_gb, ctx_w2,
           agt_w, norm_w, norm_b,
           lin_w, lin_gw, lin_gb):
    for name, arr, val in (("dist_gw", dist_gw, 1), ("dist_gb", dist_gb, 0),
                           ("q_gw", q_gw, 1), ("q_gb", q_gb, 0),
                           ("ctx_gw", ctx_gw, 1), ("ctx_gb", ctx_gb, 0),
                           ("norm_w", norm_w, 1), ("norm_b", norm_b, 0),
                           ("lin_gw", lin_gw, 1), ("lin_gb", lin_gb, 0)):
        assert np.allclose(np.asarray(arr), val), f"{name} must be trivial"

    ctx_w1 = np.asarray(ctx_w1, np.float32)
    w1 = np.asarray(dist_w1, np.float32)
    b1 = np.asarray(dist_b1, np.float32)
    w1_aug = np.zeros((4, D), np.float32)
    w1_aug[0:2] = w1
    w1_aug[2] = b1
    weights = dict(
        w1_aug=w1_aug.astype(bf16),
        dist_w2=np.asarray(dist_w2, np.float32).astype(bf16),
        q_w=np.asarray(q_w, np.float32).astype(bf16),
        Wd=np.ascontiguousarray(ctx_w1[0:D]).astype(bf16),
        Wq=np.ascontiguousarray(ctx_w1[D:2 * D]).astype(bf16),
        Wc=np.ascontiguousarray(ctx_w1[2 * D:3 * D]).astype(bf16),
        ctx_w2=np.asarray(ctx_w2, np.float32).astype(bf16),
        agt_w=np.asarray(agt_w, np.float32).astype(bf16),
        lin_w=np.asarray(lin_w, np.float32).astype(bf16),
    )

    cores, shared, T_LO, T_HI = _host_prep(agts, ctx, agt_ctrs, ctx_ctrs,
                                           hi, wi, weights)
    key = (T_LO, T_HI)
    if key not in _cached:
        _cached[key] = _build_program(T_LO, T_HI)
    nc = _cached[key]

    in_maps = []
    for k in range(NCORES):
        m = dict(cores[k])
        m.update(shared)
        in_maps.append(m)

    res = run_bass_kernel_spmd(nc, in_maps, core_ids=list(range(NCORES)),
                               **_extra_run_kwargs)
    globals()["_last_results"] = res
    out = np.concatenate([res.results[k]["out"] for k in range(NCORES)], axis=0)
    return out.astype(np.float32)


if __name__ == "__main__":
    pass



# revision 9
# speedup vs baseline: 1.5081x; 1.5081x over previous
"""Trainium2 Bass kernel for nn_Att_23313082483285 (GNN message passing).

v3 design: partition agent nodes across 8 cores (8192 each). Edges routed to
the core owning hi. No node-block grouping: per core, edges are simply split
into a lo stream (wi < 32768) and a hi stream (wi >= 32768) so the ctx
dma_gather index fits int16; tile count NT ~= ceil(E/8/128) (little padding).

GroupNorm algebra: mean-centering is folded into the weights on the host
(W -> W @ (I - J/128)), so on device each GN is: batched bn_stats/bn_aggr for
the variance, Sqrt+reciprocal for rinv, and a per-chunk
tensor_scalar(mult, max) apply (bf16 SBUF 4x mode). The node-level norm-GN
rinv cancels exactly in the following lin-GN (row-scale invariance) and is
dropped.

Per-edge data flow (feature f / edge e major):
  dT4 -> L1 matmul -> relu -> L2 (centered) -> GN1 -> h -> transpose ->
  C1 = h@Wd' + ctx[wi]@Wc' + Q2[hi] (identity mm) -> GN2 -> g ->
  dma_scatter_add into G dram by hi (message accumulation, bf16).
Q2[hi] and ctx[wi] come from dma_gather (Q2 table built in phase 1).
Phase 3 per 512 nodes: S = G@ctx_w2' + A1 -> relu -> lin_w' -> lin-GN ->
+res -> relu -> out.
"""
import sys
sys.path.insert(0, '/opt/trn_rl_repo')

import numpy as np
import ml_dtypes
from contextlib import ExitStack

from concourse import bass, mybir, tile
import concourse.bacc as bacc
from concourse.bass_utils import run_bass_kernel_spmd
from concourse.masks import make_identity

bf16 = ml_dtypes.bfloat16
P = 128
N_AGT = 65536
N_CTX = 65536
E = 400000
D = 128
EPS = 1e-5
NCORES = 8
NPC = N_AGT // NCORES          # 8192 nodes per core
NGRP = NPC // (4 * P)          # 16 groups of 512 nodes
CTX_HALF = 32768
G_TILES = 16                   # tiles per gather dma op chunk (2048 idxs)
SC_TILES = 8                   # tiles per scatter-add op. HW dma_scatter_add
                               # races on duplicate indices WITHIN one op, so
                               # the host assigns each node's edges to
                               # distinct scatter chunks.
TRASH = NPC                    # scatter target row for pad edges

f32 = mybir.dt.float32
bft = mybir.dt.bfloat16
i16 = mybir.dt.int16
Act = mybir.ActivationFunctionType
Alu = mybir.AluOpType


def _wrap16(flat_idx):
    """dma_gather/scatter idx layout: [16, n/16] wrapped, tiled x8."""
    w = flat_idx.reshape(-1, 16).T.astype(np.int16)
    return np.ascontiguousarray(np.tile(w, (8, 1)))


def _assign_chunks(hl, n_chunks, cap):
    """Order edges so no node appears twice in one scatter chunk.

    hl: per-edge node id. Returns edge positions array pos[len(hl)] giving
    the slot (chunk*cap + j) of each edge, or None if infeasible."""
    n = len(hl)
    order = np.argsort(hl, kind='stable')
    fill = np.zeros(n_chunks, np.int64)
    chunk_of = np.empty(n, np.int64)
    i = 0
    # iterate nodes in decreasing degree
    uniq, starts, counts = np.unique(hl[order], return_index=True,
                                     return_counts=True)
    node_order = np.argsort(-counts, kind='stable')
    for ni in node_order:
        k = counts[ni]
        cand = np.argpartition(fill, k - 1)[:k] if k < n_chunks else \
            np.arange(n_chunks)
        if k > n_chunks or fill[cand].max() >= cap:
            return None
        es = order[starts[ni]:starts[ni] + k]
        chunk_of[es] = cand
        fill[cand] += 1
    # slot positions
    pos = np.empty(n, np.int64)
    nxt = np.zeros(n_chunks, np.int64)
    for e in range(n):
        c = chunk_of[e]
        pos[e] = c * cap + nxt[c]
        nxt[c] += 1
    return pos


def _host_prep(agts, ctx, agt_ctrs, ctx_ctrs, hi, wi):
    """Route edges per core (lo/hi ctx stream), build per-core arrays."""
    hi = np.asarray(hi).astype(np.int64)
    wi = np.asarray(wi).astype(np.int64)
    agts = np.asarray(agts, dtype=np.float32)
    d_all = (np.asarray(agt_ctrs, np.float32)[hi]
             - np.asarray(ctx_ctrs, np.float32)[wi])          # [E, 2]

    core = hi // NPC
    is_hi = wi >= CTX_HALF

    # uniform tile counts across cores (SPMD)
    nlo_max = 0
    nhi_max = 0
    per_core = []
    for k in range(NCORES):
        m = core == k
        lo_e = np.nonzero(m & ~is_hi)[0]
        hi_e = np.nonzero(m & is_hi)[0]
        per_core.append((lo_e, hi_e))
        nlo_max = max(nlo_max, len(lo_e))
        nhi_max = max(nhi_max, len(hi_e))
    NLO_T = -(-(-(-nlo_max // P)) // SC_TILES) * SC_TILES  # mult of SC_TILES
    NHI_T = -(-(-(-nhi_max // P)) // SC_TILES) * SC_TILES
    # scatter-chunk feasibility: per stream, max node degree <= n_chunks and
    # greedy assignment must fit; bump tiles until it works
    cap = SC_TILES * P
    pos_all = {}
    for k in range(NCORES):
        for half, edges in enumerate(per_core[k]):
            hl = hi[edges] - k * NPC
            while True:
                n_t = NLO_T if half == 0 else NHI_T
                pos = _assign_chunks(hl, n_t // SC_TILES, cap)
                if pos is not None:
                    break
                if half == 0:
                    NLO_T += SC_TILES
                else:
                    NHI_T += SC_TILES
            pos_all[(k, half)] = pos
    NT = NLO_T + NHI_T

    cores = []
    for k in range(NCORES):
        lo_e, hi_e = per_core[k]
        dT4 = np.zeros((4, NT * P), np.float32)
        qflat = np.zeros(NT * P, np.int64)          # Q2 gather idx (pad -> 0)
        sflat = np.full(NT * P, TRASH, np.int64)    # scatter idx (pad -> trash)
        lo_flat = np.zeros(NLO_T * P, np.int64)
        hi_flat = np.zeros(NHI_T * P, np.int64)

        for half, (edges, base, flat) in enumerate(
                ((lo_e, 0, lo_flat), (hi_e, NLO_T * P, hi_flat))):
            pos = pos_all[(k, half)]
            cols = base + pos
            dT4[0, cols] = d_all[edges, 0]
            dT4[1, cols] = d_all[edges, 1]
            dT4[2, cols] = 1.0
            hl = hi[edges] - k * NPC
            qflat[cols] = hl
            sflat[cols] = hl
            flat[pos] = wi[edges] if half == 0 else wi[edges] - CTX_HALF

        cores.append(dict(
            agtsT=np.ascontiguousarray(agts[k * NPC:(k + 1) * NPC].T.astype(bf16)),
            resb=np.ascontiguousarray(agts[k * NPC:(k + 1) * NPC].astype(bf16)),
            dT4=dT4.astype(bf16),
            qidx=_wrap16(qflat),
            sidx=_wrap16(sflat),
            widx_lo=_wrap16(lo_flat),
            widx_hi=_wrap16(hi_flat),
        ))
    return cores, NLO_T, NHI_T


def _build_program(NLO_T, NHI_T):
    NT = NLO_T + NHI_T
    NST = NT // 4

    nc = bacc.Bacc("TRN2", target_bir_lowering=False, debug=False,
                   enable_asserts=False, num_devices=NCORES,
                   dynamic_dma_scratch_size=16384)

    def din(name, shape, dt):
        return nc.dram_tensor(name, list(shape), dt, kind="ExternalInput").ap()

    t_agtsT = din("agtsT", (P, NPC), bft)
    t_resb = din("resb", (NPC, D), bft)
    t_ctx = din("ctx_bf", (N_CTX, D), bft)
    t_dT4 = din("dT4", (4, NT * P), bft)
    t_qidx = din("qidx", (P, NT * P // 16), i16)
    t_sidx = din("sidx", (P, NT * P // 16), i16)
    t_wlo = din("widx_lo", (P, NLO_T * P // 16), i16)
    t_whi = din("widx_hi", (P, NHI_T * P // 16), i16)
    wnames = ["w1_aug", "W2c", "Wdc", "Wcc", "Wqc", "qwc",
              "agtwc", "ctxw2c", "linwc"]
    t_w = {n: din(n, (4, D) if n == "w1_aug" else (D, D), bft) for n in wnames}
    t_out = nc.dram_tensor("out", [NPC, D], bft, kind="ExternalOutput").ap()

    with tile.TileContext(nc) as tc, ExitStack() as ctx:
        const = ctx.enter_context(tc.tile_pool(name="const", bufs=1))
        big = ctx.enter_context(tc.tile_pool(name="big", bufs=1))
        dram = ctx.enter_context(tc.tile_pool(name="dram", bufs=1, space="DRAM"))
        sb = ctx.enter_context(tc.tile_pool(name="sb", bufs=4))
        gb = ctx.enter_context(tc.tile_pool(name="gb", bufs=3))
        ps = ctx.enter_context(tc.tile_pool(name="ps", bufs=8, space="PSUM"))

        # ---------- constants ----------
        ident = const.tile([P, P], f32)
        make_identity(nc, ident[:])
        ident_bf = const.tile([P, P], bft)
        nc.vector.tensor_copy(ident_bf[:], ident[:])
        eps_t = const.tile([P, 1], f32)
        nc.gpsimd.memset(eps_t[:], EPS)
        zt = const.tile([P, 4, D], bft)
        nc.gpsimd.memset(zt[:], 0.0)
        w_sb = {}
        for n in wnames:
            shp = [4, D] if n == "w1_aug" else [D, D]
            w_sb[n] = const.tile(shp, bft, name=f"w_{n}")
            nc.sync.dma_start(w_sb[n][:], t_w[n][:])

        # big resident tensors
        agtsT = big.tile([P, NPC], bft)
        nc.sync.dma_start(agtsT[:], t_agtsT[:])
        qidx = big.tile([P, NT * P // 16], i16)
        nc.sync.dma_start(qidx[:], t_qidx[:])
        sidx = big.tile([P, NT * P // 16], i16)
        nc.sync.dma_start(sidx[:], t_sidx[:])
        wlo = big.tile([P, NLO_T * P // 16], i16)
        nc.sync.dma_start(wlo[:], t_wlo[:])
        whi = big.tile([P, NHI_T * P // 16], i16)
        nc.sync.dma_start(whi[:], t_whi[:])
        A1b = big.tile([P, NPC // P, D], bft)

        Q2d = dram.tile([NPC, D], bft)
        Gd = dram.tile([NPC + P, D], bft)

        # helper: GN variance -> rinv [P,4] f32. Mean is folded into the
        # pre-centered weights; bn_stats per chunk (tensor_tensor_reduce
        # crashes the device, bn_stats is HW-proven).
        def gn_rinv(src_b, tag, stats_eng):
            bn6 = sb.tile([P, 4, 6], f32, tag=f"bn6{tag}")
            for c in range(4):
                stats_eng.bn_stats(bn6[:, c, :], src_b[:, c, :])
            bn2 = sb.tile([P, 4, 2], f32, tag=f"bn2{tag}")
            for c in range(4):
                stats_eng.bn_aggr(bn2[:, c, :], bn6[:, c, :])
            sd = sb.tile([P, 4], f32, tag=f"sd{tag}")
            nc.scalar.activation(sd[:], bn2[:, :, 1], Act.Sqrt, bias=eps_t[:])
            rinv = sb.tile([P, 4], f32, tag=f"ri{tag}")
            nc.vector.reciprocal(rinv[:], sd[:])
            return rinv

        # ---------- phase 1: node-level precompute ----------
        # A1 = agts @ agt_w' (bf16, SBUF); Q2 = (r_q*relu(agts@q_w')) @ Wq' -> DRAM
        for g in range(NGRP):
            ps_a = ps.tile([P, 4, D], f32, space="PSUM", tag="pp")
            for c in range(4):
                j = g * 4 + c
                nc.tensor.matmul(ps_a[:, c, :], lhsT=agtsT[:, j * P:(j + 1) * P],
                                 rhs=w_sb["agtwc"][:], start=True, stop=True)
            nc.scalar.copy(A1b[:, g * 4:(g + 1) * 4, :], ps_a[:])

            ps_q = ps.tile([P, 4, D], f32, space="PSUM", tag="pp")
            for c in range(4):
                j = g * 4 + c
                nc.tensor.matmul(ps_q[:, c, :], lhsT=agtsT[:, j * P:(j + 1) * P],
                                 rhs=w_sb["qwc"][:], start=True, stop=True)
            yqb = sb.tile([P, 4, D], bft, tag="yqb")
            nc.scalar.copy(yqb[:], ps_q[:])
            r_q = gn_rinv(yqb, "q", nc.vector)
            qn = sb.tile([P, 4, D], bft, tag="qn")
            for c in range(4):
                nc.vector.tensor_scalar(qn[:, c, :], yqb[:, c, :],
                                        r_q[:, c:c + 1], 0.0,
                                        op0=Alu.mult, op1=Alu.max)
            ps_t = ps.tile([P, 4, D], bft, space="PSUM", tag="pp")
            for c in range(4):
                nc.tensor.transpose(ps_t[:, c, :], qn[:, c, :], ident_bf[:])
            qnT = sb.tile([P, 4, D], bft, tag="qnT")
            nc.vector.tensor_copy(qnT[:], ps_t[:])
            ps_q2 = ps.tile([P, 4, D], f32, space="PSUM", tag="pp")
            for c in range(4):
                nc.tensor.matmul(ps_q2[:, c, :], lhsT=qnT[:, c, :],
                                 rhs=w_sb["Wqc"][:], start=True, stop=True)
            q2b = sb.tile([P, 4, D], bft, tag="q2b")
            nc.scalar.copy(q2b[:], ps_q2[:])
            nc.sync.dma_start(
                Q2d[g * 512:(g + 1) * 512, :].rearrange(
                    "(c p) f -> p c f", p=P),
                q2b[:])
            # zero-init message accumulator G
            nc.sync.dma_start(
                Gd[g * 512:(g + 1) * 512, :].rearrange(
                    "(c p) f -> p c f", p=P),
                zt[:])
        nc.sync.dma_start(Gd[NPC:NPC + P, :], zt[:, 0, :])

        # ---------- phase 2: edge pipeline ----------
        dt_bufs = {}
        q2_bufs = {}
        lo_bufs = {}
        hi_bufs = {}
        gst_bufs = {}

        def issue_dt(gi):
            nt = min(G_TILES, NT - gi * G_TILES)
            buf = gb.tile([4, G_TILES * P], bft, tag="dt4")
            nc.sync.dma_start(buf[:, :nt * P],
                              t_dT4[:, gi * G_TILES * P:(gi * G_TILES + nt) * P])
            dt_bufs[gi] = buf

        def issue_q2(gi):
            nt = min(G_TILES, NT - gi * G_TILES)
            buf = gb.tile([P, G_TILES, D], bft, tag="q2g")
            nc.gpsimd.dma_gather(
                out_ap=buf[:, :nt, :], in_ap=Q2d[:],
                idxs_ap=qidx[:, gi * G_TILES * 8:(gi * G_TILES + nt) * 8],
                num_idxs=nt * P, num_idxs_reg=nt * P, elem_size=D,
                single_packet=False)
            q2_bufs[gi] = buf

        def issue_w(gi, half):
            n_str, src, idxt, bufs, tag = (
                (NLO_T, t_ctx[:CTX_HALF, :], wlo, lo_bufs, "clo") if half == 0
                else (NHI_T, t_ctx[CTX_HALF:, :], whi, hi_bufs, "chi"))
            nt = min(G_TILES, n_str - gi * G_TILES)
            buf = gb.tile([P, 1, G_TILES * P], bft, tag=tag)
            nc.gpsimd.dma_gather(
                out_ap=buf[:, :, :nt * P], in_ap=src,
                idxs_ap=idxt[:, gi * G_TILES * 8:(gi * G_TILES + nt) * 8],
                num_idxs=nt * P, num_idxs_reg=nt * P, elem_size=D,
                transpose=True, single_packet=False)
            bufs[gi] = buf

        def tmeta(t):
            return (0, t) if t < NLO_T else (1, t - NLO_T)

        def st_stages(s):
            """Pipeline for super-tile s (4 tiles), stage-interleaved."""
            tiles = [4 * s + c for c in range(4)]
            gi0, off0 = divmod(4 * s, G_TILES)
            # stage 0: gathers / loads
            sc0, soff0 = divmod(4 * s, SC_TILES)
            if gi0 not in dt_bufs:
                issue_dt(gi0)
            if gi0 not in q2_bufs:
                issue_q2(gi0)
            if sc0 not in gst_bufs:
                gst_bufs[sc0] = gb.tile([P, SC_TILES, D], bft, tag="gst",
                                        name="gst")
            for t in tiles:
                half, si = tmeta(t)
                gi = si // G_TILES
                if half == 0 and gi not in lo_bufs:
                    issue_w(gi, 0)
                if half == 1 and gi not in hi_bufs:
                    issue_w(gi, 1)
            yield
            # stage 1: L1  y1T [f, 512] = w1_aug.T @ dT4
            ps_y1 = ps.tile([P, 4 * D], f32, space="PSUM", tag="pp")
            nc.tensor.matmul(ps_y1[:], lhsT=w_sb["w1_aug"][:],
                             rhs=dt_bufs[gi0][:, off0 * P:(off0 + 4) * P],
                             start=True, stop=True)
            yield
            # stage 2: relu -> r1T bf16
            r1T = sb.tile([P, 4 * D], bft, tag="r1T")
            nc.scalar.activation(r1T[:], ps_y1[:], Act.Relu)
            yield
            # stage 3: L2 (centered) -> ps2 edge-major
            ps2 = ps.tile([P, 4, D], f32, space="PSUM", tag="pp")
            for c in range(4):
                nc.tensor.matmul(ps2[:, c, :], lhsT=r1T[:, c * D:(c + 1) * D],
                                 rhs=w_sb["W2c"][:], start=True, stop=True)
            yield
            # stage 4: copy to bf16 SBUF
            y2b = sb.tile([P, 4, D], bft, tag="y2b")
            nc.scalar.copy(y2b[:], ps2[:])
            yield
            # stage 5: GN1 stats
            r1e = gn_rinv(y2b, "d", nc.vector)
            yield
            # stage 6: GN1 apply -> h
            h = sb.tile([P, 4, D], bft, tag="h")
            for c in range(4):
                nc.vector.tensor_scalar(h[:, c, :], y2b[:, c, :],
                                        r1e[:, c:c + 1], 0.0,
                                        op0=Alu.mult, op1=Alu.max)
            yield
            # stage 7: T(h)
            psT = ps.tile([P, 4, D], bft, space="PSUM", tag="pp")
            for c in range(4):
                nc.tensor.transpose(psT[:, c, :], h[:, c, :], ident_bf[:])
            yield
            # stage 8: hT copy
            hT = sb.tile([P, 4, D], bft, tag="hT")
            nc.vector.tensor_copy(hT[:], psT[:])
            yield
            # stage 9: C1 = h@Wd' + ctx[wi]@Wc' + Q2[hi]
            ps3 = ps.tile([P, 4, D], f32, space="PSUM", tag="pp")
            for c, t in enumerate(tiles):
                half, si = tmeta(t)
                gi, off = divmod(si, G_TILES)
                cbuf = lo_bufs[gi] if half == 0 else hi_bufs[gi]
                qgi, qoff = divmod(t, G_TILES)
                nc.tensor.matmul(ps3[:, c, :], lhsT=hT[:, c, :],
                                 rhs=w_sb["Wdc"][:], start=True, stop=False)
                nc.tensor.matmul(ps3[:, c, :],
                                 lhsT=cbuf[:, 0, off * P:(off + 1) * P],
                                 rhs=w_sb["Wcc"][:], start=False, stop=False)
                nc.tensor.matmul(ps3[:, c, :], lhsT=ident_bf[:],
                                 rhs=q2_bufs[qgi][:, qoff, :],
                                 start=False, stop=True)
            yield
            # stage 10: copy to bf16
            g0b = sb.tile([P, 4, D], bft, tag="g0b")
            nc.scalar.copy(g0b[:], ps3[:])
            yield
            # stage 11: GN2 stats
            r2 = gn_rinv(g0b, "c", nc.vector)
            yield
            # stage 12: GN2 apply -> g into scatter staging
            gst = gst_bufs[sc0]
            for c in range(4):
                nc.vector.tensor_scalar(gst[:, soff0 + c, :], g0b[:, c, :],
                                        r2[:, c:c + 1], 0.0,
                                        op0=Alu.mult, op1=Alu.max)
            yield
            # stage 13: scatter-add chunk when staging is full
            if soff0 + 4 == SC_TILES:
                nc.gpsimd.dma_scatter_add(
                    out_ap=Gd[:], in_ap=gst[:, :, :],
                    idxs_ap=sidx[:, sc0 * SC_TILES * 8:(sc0 + 1) * SC_TILES * 8],
                    num_idxs=SC_TILES * P, num_idxs_reg=SC_TILES * P,
                    elem_size=D, single_packet=False)
            yield

        ILV = 8
        grp = 0
        while grp * ILV < NST:
            n = min(ILV, NST - grp * ILV)
            gens = [st_stages(grp * ILV + j) for j in range(n)]
            alive = True
            while alive:
                alive = False
                for g_ in gens:
                    try:
                        next(g_)
                        alive = True
                    except StopIteration:
                        pass
            grp += 1

        # ---------- phase 3: node finale ----------
        for g in range(NGRP):
            Gsb = sb.tile([P, 4, D], bft, tag="Gsb")
            nc.sync.dma_start(
                Gsb[:],
                Gd[g * 512:(g + 1) * 512, :].rearrange("(c p) f -> p c f", p=P))
            psTG = ps.tile([P, 4, D], bft, space="PSUM", tag="pp")
            for c in range(4):
                nc.tensor.transpose(psTG[:, c, :], Gsb[:, c, :], ident_bf[:])
            GT = sb.tile([P, 4, D], bft, tag="GT")
            nc.vector.tensor_copy(GT[:], psTG[:])
            ps_S = ps.tile([P, 4, D], f32, space="PSUM", tag="pp")
            for c in range(4):
                nc.tensor.matmul(ps_S[:, c, :], lhsT=GT[:, c, :],
                                 rhs=w_sb["ctxw2c"][:], start=True, stop=False)
                nc.tensor.matmul(ps_S[:, c, :], lhsT=ident_bf[:],
                                 rhs=A1b[:, g * 4 + c, :],
                                 start=False, stop=True)
            # norm-GN: centered via weights; rinv cancels in lin-GN -> relu only
            o1u = sb.tile([P, 4, D], bft, tag="o1u")
            nc.scalar.activation(o1u[:], ps_S[:], Act.Relu)
            psT2 = ps.tile([P, 4, D], bft, space="PSUM", tag="pp")
            for c in range(4):
                nc.tensor.transpose(psT2[:, c, :], o1u[:, c, :], ident_bf[:])
            o1T = sb.tile([P, 4, D], bft, tag="o1T")
            nc.vector.tensor_copy(o1T[:], psT2[:])
            ps_l = ps.tile([P, 4, D], f32, space="PSUM", tag="pp")
            for c in range(4):
                nc.tensor.matmul(ps_l[:, c, :], lhsT=o1T[:, c, :],
                                 rhs=w_sb["linwc"][:], start=True, stop=True)
            zb = sb.tile([P, 4, D], bft, tag="zb")
            nc.scalar.copy(zb[:], ps_l[:])
            r_l = gn_rinv(zb, "l", nc.vector)
            o2 = sb.tile([P, 4, D], bft, tag="o2")
            for c in range(4):
                nc.vector.tensor_scalar_mul(o2[:, c, :], zb[:, c, :],
                                            r_l[:, c:c + 1])
            res_sb = sb.tile([P, 4, D], bft, tag="res_sb")
            nc.sync.dma_start(
                res_sb[:],
                t_resb[g * 512:(g + 1) * 512, :].rearrange(
                    "(c p) f -> p c f", p=P))
            fin = sb.tile([P, 4, D], bft, tag="fin")
            nc.vector.tensor_tensor(out=fin[:], in0=o2[:], in1=res_sb[:],
                                    op=Alu.add)
            nc.vector.tensor_scalar_max(fin[:], fin[:], 0.0)
            nc.sync.dma_start(
                t_out[g * 512:(g + 1) * 512, :].rearrange(
                    "(c p) f -> p c f", p=P),
                fin[:])

    nc.compile()
    return nc


_cached = {}
_extra_run_kwargs = {}
_last_results = None


def run_traced(inputs):
    global _extra_run_kwargs
    _extra_run_kwargs = dict(trace=True)
    try:
        kernel(**inputs)
    finally:
        _extra_run_kwargs = {}
    return _last_results


def kernel(agts, ctx, agt_ctrs, ctx_ctrs, hi, wi,
           dist_w1, dist_b1, dist_w2, dist_gw, dist_gb,
           q_w, q_gw, q_gb,
           ctx_w1, ctx_gw, ctx_gb, ctx_w2,
           agt_w, norm_w, norm_b,
           lin_w, lin_gw, lin_gb):
    for name, arr, val in (("dist_gw", dist_gw, 1), ("dist_gb", dist_gb, 0),
                           ("q_gw", q_gw, 1), ("q_gb", q_gb, 0),
                           ("ctx_gw", ctx_gw, 1), ("ctx_gb", ctx_gb, 0),
                           ("norm_w", norm_w, 1), ("norm_b", norm_b, 0),
                           ("lin_gw", lin_gw, 1), ("lin_gb", lin_gb, 0)):
        assert np.allclose(np.asarray(arr), val), f"{name} must be trivial"

    C = np.eye(D, dtype=np.float64) - 1.0 / D   # GN mean-centering projector
    ctx_w1 = np.asarray(ctx_w1, np.float64)
    w1 = np.asarray(dist_w1, np.float32)
    b1 = np.asarray(dist_b1, np.float32)
    w1_aug = np.zeros((4, D), np.float32)
    w1_aug[0:2] = w1
    w1_aug[2] = b1
    weights = dict(
        w1_aug=w1_aug.astype(bf16),
        W2c=(np.asarray(dist_w2, np.float64) @ C).astype(bf16),
        Wdc=(ctx_w1[0:D] @ C).astype(bf16),
        Wqc=(ctx_w1[D:2 * D] @ C).astype(bf16),
        Wcc=(ctx_w1[2 * D:3 * D] @ C).astype(bf16),
        qwc=(np.asarray(q_w, np.float64) @ C).astype(bf16),
        agtwc=(np.asarray(agt_w, np.float64) @ C).astype(bf16),
        ctxw2c=(np.asarray(ctx_w2, np.float64) @ C).astype(bf16),
        linwc=(np.asarray(lin_w, np.float64) @ C).astype(bf16),
    )

    cores, NLO_T, NHI_T = _host_prep(agts, ctx, agt_ctrs, ctx_ctrs, hi, wi)
    key = (NLO_T, NHI_T)
    if key not in _cached:
        _cached[key] = _build_program(NLO_T, NHI_T)
    nc = _cached[key]

    shared = dict(ctx_bf=np.ascontiguousarray(
        np.asarray(ctx, np.float32).astype(bf16)), **weights)
    in_maps = []
    for k in range(NCORES):
        m = dict(cores[k])
        m.update(shared)
        in_maps.append(m)

    res = run_bass_kernel_spmd(nc, in_maps, core_ids=list(range(NCORES)),
                               **_extra_run_kwargs)
    globals()["_last_results"] = res
    out = np.concatenate([res.results[k]["out"] for k in range(NCORES)], axis=0)
    return out.astype(np.float32)


if __name__ == "__main__":
    pass
